# revision 1
# baseline (speedup 1.0000x reference)
# Trainium2 Bass kernel for nn_MultiHeadAttention_48533130445634.
#
# Math (faithful to the reference, including its unusual second einsum):
#   scores[b,h,n,m] = softmax_m( (q[b,h,n,:] . k[b,h,m,:]) * 0.125 )
#   out[b,h,m,d]    = (sum_n scores[b,h,n,m]) * v[b,h,m,d]
#
# i.e. the output is V scaled elementwise by the column-sums of the softmax
# matrix.  Per (b,h), tiled over n (128 rows at a time):
#   S_i = Q_i K^T            (PE, fp32r, PSUM out, 1024-wide halves)
#   E_i = exp(S_i * 0.125)   (ACT, bf16 out to SBUF; some tiles also emit
#                             the row-sum via the ACT accumulator)
#   rowsum_i                 (ACT accum or DVE free-axis reduce over E_i)
#   g_i = 1 / rowsum_i       (DVE reciprocal, batched)
#   colsum += g_i^T @ E_i    (PE, bf16; accumulated in one PSUM bank using
#                             four output base-partitions 0/32/64/96, one
#                             per 512-wide m-chunk)
#   out[m,d] = colsum[m] * v[m,d]   (GpSimd tensor_scalar)
#
# The exp pass on the scalar engine is the roofline (~270us/core); the PE
# (QK^T + colsum) and DVE (rowsum reduces) run just below it.  Colsum
# matmuls are queued and drip-fed (one tile's worth per n-tile slot, only
# once their g is ready) so they never head-of-line-block the S fills that
# feed the ACT.  The m index maps to partitions as m = 16p + t so V loads,
# output stores and the colsum scatter are all contiguous per partition.
#
# Sharding: 64 (b,h) pairs split across 8 cores, 8 pairs each (SPMD, no
# cross-core communication).  Q/K are pre-transposed on the host so the
# contraction dim (Dh=64) lands on SBUF partitions for the PE.

import os

import numpy as np

import concourse.mybir as mybir
import concourse.tile as tile
from concourse import bacc
from concourse.bass_utils import run_bass_kernel_spmd

B, H, N, D = 4, 16, 2048, 64
N_CORES = 8
H_LOC = (B * H) // N_CORES  # 8 (b,h) pairs per core
P = 128                     # partition tile along n
NT = N // P                 # 16 n-tiles
SCALE = 0.125               # (DIM // N_HEADS) ** -0.5
MH = 2                      # m processed in halves of 1024 (PSUM bank budget)
MW = N // MH                # 1024

# n-tiles (within a head) whose rowsum comes from the ACT accumulator; the
# rest use a DVE reduce over E_i.  These are the last tiles of the first
# three g-batches, so each batch's g is ready as soon as its last exp ends.
ACCUM_TILES = frozenset({3, 7, 11, 15})

f32 = mybir.dt.float32
f32r = mybir.dt.float32r
bf16 = mybir.dt.bfloat16
Exp = mybir.ActivationFunctionType.Exp


def _attention_kernel(tc, out, qT, kT, vin):
    nc = tc.nc

    with (
        tc.tile_pool(name="qk", bufs=3) as qk_pool,
        tc.tile_pool(name="ev", bufs=14) as e_pool,
        tc.tile_pool(name="vo", bufs=4) as vo_pool,
        tc.tile_pool(name="st", bufs=2) as st_pool,
        tc.tile_pool(name="s_ps", bufs=3, space="PSUM") as s_pool,
        tc.tile_pool(name="c_ps", bufs=2, space="PSUM") as c_pool,
    ):
        # Preload the exp table set and start the PE p-state ramp while the
        # first DMAs are in flight.
        warm = st_pool.tile([P, 1], f32, tag="warm")
        nc.gpsimd.memset(warm[:, :], 0.0)
        nc.scalar.activation(warm[:, :], warm[:, :], func=Exp)
        warm_ps = c_pool.tile([1, 1], f32, tag="csum")
        nc.tensor.matmul(
            warm_ps[0:1, 0:1], lhsT=warm[0:1, 0:1], rhs=warm[0:1, 0:1],
            start=True, stop=True, skip_group_check=True,
        )

        # Q/K/V loads for head h, emitted one head ahead so the SP sequencer
        # issues them before it blocks on the previous head's tail DMAs.
        loaded = {}

        def emit_loads(h, first=False):
            q_s = qk_pool.tile([D, N], f32r, tag="q")
            k_s = qk_pool.tile([D, N], f32r, tag="k")
            if first:
                # order so the very first S fill's operands land earliest
                parts = [(k_s, kT, 0, 512), (q_s, qT, 0, 512),
                         (k_s, kT, 512, MW), (k_s, kT, MW, N),
                         (q_s, qT, 512, MW), (q_s, qT, MW, N)]
                for t_s, src, lo, hi in parts:
                    nc.sync.dma_start(out=t_s[:, lo:hi], in_=src[h, :, lo:hi])
            else:
                for half in range(2):
                    sl = slice(half * MW, (half + 1) * MW)
                    nc.sync.dma_start(out=k_s[:, sl], in_=kT[h, :, sl])
                    nc.sync.dma_start(out=q_s[:, sl], in_=qT[h, :, sl])
            v_s = vo_pool.tile([P, NT, D], f32, tag="v")
            nc.sync.dma_start(
                out=v_s[:, :, :], in_=vin[h].rearrange("(p t) d -> p t d", p=P)
            )
            # prepare the colsum accumulator a head ahead too, so its
            # memset (DVE) and PSUM WAR never stall the head boundary
            c_ps = c_pool.tile([P, 512], f32, tag="csum")
            nc.vector.memset(c_ps[:, :], 0.0)
            loaded[h] = (q_s, k_s, v_s, c_ps)

        emit_loads(0, first=True)

        # colsum matmuls pending emission: (min_slot, j, c_ps, g_bf, e_j, tail)
        pending = []
        slot = 0

        def emit_colsum(entry, c_lo, c_hi):
            _, j, c_ps, g_bf, e_j, tail_fn = entry
            for c in range(c_lo, c_hi):
                nc.tensor.matmul(
                    c_ps[32 * c : 32 * c + 1, :],
                    lhsT=g_bf[:, j : j + 1],
                    rhs=e_j[:, c * 512 : (c + 1) * 512],
                    start=(j == 0),
                    stop=(j == NT - 1),
                    skip_group_check=True,
                    tile_position=(0, 32 * c),
                )
            if c_hi == N // 512 and tail_fn is not None:
                tail_fn()

        for h in range(H_LOC):
            last_head = h == H_LOC - 1
            q_s, k_s, v_s, c_ps = loaded.pop(h)
            if not last_head:
                emit_loads(h + 1)

            # per-tile rowsum parts: [:, i, 0] and [:, i, 1] summed later
            rs_parts = st_pool.tile([P, NT, 2], f32, tag="rsp")
            nc.gpsimd.memset(rs_parts[:, :, :], 0.0)
            rowsum = st_pool.tile([P, NT], f32, tag="rowsum")
            g = st_pool.tile([P, NT], f32, tag="g")
            g_bf = st_pool.tile([P, NT], bf16, tag="gbf")
            e_tiles = []

            def make_tail(h=h, c_ps=c_ps, v_s=v_s, last_head=last_head):
                def tail():
                    # colsum [4 x 512 at partitions 0/32/64/96] -> csT [P, NT].
                    # m = 16p + t, so chunk-major cs4 [4x512] and csT [128,16]
                    # walk m in the same order: one direct SBUF->SBUF DMA.
                    cs4 = st_pool.tile([P, 512], f32, tag="cs4")
                    nc.vector.tensor_copy(cs4[:, :], c_ps[:, :])
                    csT = st_pool.tile([P, NT], f32, tag="csT")
                    nc.sync.dma_start(out=csT[:, :], in_=cs4[0:P:32, :])
                    o_s = vo_pool.tile([P, NT, D], f32, tag="o")
                    eng = nc.vector if last_head else nc.gpsimd
                    out_r = out[h].rearrange("(p t) d -> p t d", p=P)
                    halves = ((0, NT // 2), (NT // 2, NT)) if last_head else ((0, NT),)
                    for t0, t1 in halves:
                        eng.tensor_tensor(
                            o_s[:, t0:t1, :],
                            v_s[:, t0:t1, :],
                            csT[:, t0:t1].unsqueeze(-1).broadcast_to((P, t1 - t0, D)),
                            op=mybir.AluOpType.mult,
                        )
                        nc.sync.dma_start(
                            out=out_r[:, t0:t1, :], in_=o_s[:, t0:t1, :]
                        )

                return tail

            tail_fn = make_tail()

            # g-batches: (first_tile, last_tile inclusive); the last head
            # finishes with micro-batches so colsum work drains in-loop
            if last_head:
                batches = [(0, 3), (4, 7), (8, 11), (12, 13), (14, 14), (15, 15)]
            else:
                batches = [(b0, b0 + 3) for b0 in range(0, NT, 4)]
            batch_of = {}
            for b0, b1 in batches:
                for j in range(b0, b1 + 1):
                    batch_of[j] = (b0, b1)

            for i in range(NT):
                slot += 1
                e_i = e_pool.tile([P, N], bf16, tag="e")
                e_tiles.append(e_i)
                use_accum = (i in ACCUM_TILES) or (last_head and i >= 13)
                # drip-feed pending colsum matmuls: half a tile's worth
                # after each m-half, so PE bursts never delay the S fills
                entry = None
                for mh in range(MH):
                    s_ps = s_pool.tile([P, MW], f32, tag="s")
                    for c in range(MW // 512):
                        m0 = mh * MW + c * 512
                        nc.tensor.matmul(
                            s_ps[:, c * 512 : (c + 1) * 512],
                            lhsT=q_s[:, i * P : (i + 1) * P],
                            rhs=k_s[:, m0 : m0 + 512],
                            start=True,
                            stop=True,
                        )
                    nc.scalar.activation(
                        e_i[:, mh * MW : (mh + 1) * MW],
                        s_ps[:, :],
                        func=Exp,
                        scale=SCALE,
                        accum_out=rs_parts[:, i, mh : mh + 1] if use_accum else None,
                    )
                    if mh == 0:
                        if pending and pending[0][0] <= slot:
                            entry = pending.pop(0)
                        if entry is not None:
                            emit_colsum(entry, 0, 2)
                    elif entry is not None:
                        emit_colsum(entry, 2, 4)
                if not use_accum:
                    nc.vector.tensor_reduce(
                        rs_parts[:, i, 0:1],
                        e_i[:, :],
                        axis=mybir.AxisListType.X,
                        op=mybir.AluOpType.add,
                    )

                if i == batch_of[i][1]:  # batch boundary: g for the batch
                    b0, b1 = batch_of[i]
                    sl = slice(b0, b1 + 1)
                    nc.vector.tensor_tensor(
                        rowsum[:, sl],
                        rs_parts[:, sl, 0],
                        rs_parts[:, sl, 1],
                        op=mybir.AluOpType.add,
                    )
                    nc.vector.reciprocal(g[:, sl], rowsum[:, sl])
                    nc.vector.tensor_copy(g_bf[:, sl], g[:, sl])
                    # lag 1 slot if g is ready at the batch's last exp (ACT
                    # accum), else 3 slots for the DVE reduce to land.
                    lag = 1 if use_accum else 2
                    for idx, j in enumerate(range(b0, b1 + 1)):
                        pending.append(
                            (
                                slot + lag + idx,
                                j,
                                c_ps,
                                g_bf,
                                e_tiles[j],
                                tail_fn if j == NT - 1 else None,
                            )
                        )

            if last_head:
                while pending:
                    emit_colsum(pending.pop(0), 0, N // 512)


_NC_CACHE = None


def _get_nc():
    global _NC_CACHE
    if _NC_CACHE is None:
        nc = bacc.Bacc("TRN2", target_bir_lowering=False, debug=False)
        qT = nc.dram_tensor("qT", [H_LOC, D, N], f32r, kind="ExternalInput").ap()
        kT = nc.dram_tensor("kT", [H_LOC, D, N], f32r, kind="ExternalInput").ap()
        vin = nc.dram_tensor("v", [H_LOC, N, D], f32, kind="ExternalInput").ap()
        out = nc.dram_tensor("out", [H_LOC, N, D], f32, kind="ExternalOutput").ap()
        with tile.TileContext(nc) as tc:
            _attention_kernel(tc, out, qT, kT, vin)
        nc.compile()
        _NC_CACHE = nc
    return _NC_CACHE


def kernel(q, k, v):
    q = np.asarray(q, dtype=np.float32).reshape(B * H, N, D)
    k = np.asarray(k, dtype=np.float32).reshape(B * H, N, D)
    v = np.asarray(v, dtype=np.float32).reshape(B * H, N, D)

    in_maps = []
    for c in range(N_CORES):
        sl = slice(H_LOC * c, H_LOC * (c + 1))
        in_maps.append(
            {
                "qT": np.ascontiguousarray(q[sl].transpose(0, 2, 1)),
                "kT": np.ascontiguousarray(k[sl].transpose(0, 2, 1)),
                "v": np.ascontiguousarray(v[sl]),
            }
        )

    trace = bool(os.environ.get("KERNEL_TRACE"))
    res = run_bass_kernel_spmd(
        _get_nc(), in_maps, core_ids=list(range(N_CORES)), trace=trace
    )
    if trace:
        print(f"HW exec time: {res.exec_time_ns} ns")
        if res.instructions_and_trace is not None:
            print(f"trace: {res.instructions_and_trace[1]}")

    outs = [r["out"] for r in res.results]
    return np.concatenate(outs, axis=0).reshape(B, H, N, D)



# revision 3
# speedup vs baseline: 1.1341x; 1.1341x over previous
# Trainium2 Bass kernel for nn_MultiHeadAttention_48533130445634.
#
# Math (faithful to the reference, including its unusual second einsum):
#   scores[b,h,n,m] = softmax_m( (q[b,h,n,:] . k[b,h,m,:]) * 0.125 )
#   out[b,h,m,d]    = (sum_n scores[b,h,n,m]) * v[b,h,m,d]
#
# i.e. the output is V scaled elementwise by the column-sums of the softmax
# matrix.  Per (b,h), tiled over n (128 rows at a time):
#   S_i = Q_i K^T            (PE, f32r, PSUM out, 1024-wide halves)
#   E_i = exp(S_i * 0.125)   (bf16 out to SBUF; the roofline.  Split across
#                             engines: even tiles on the ACT (native Exp,
#                             rowsum via the ACT accumulator), odd tiles on
#                             the DVE via a custom op computing p(s)^8 with
#                             a fitted degree-2 p — rowsum via the op's
#                             accum output.  Softmax's ratio cancels the
#                             poly approximation's common-mode error and
#                             the colsum averages the rest: measured
#                             end-to-end rel err ~2e-3.)
#   g_i = 1 / rowsum_i       (DVE reciprocal, batched)
#   colsum += g_i^T @ E_i    (PE, bf16; accumulated in one PSUM bank using
#                             four output base-partitions 0/32/64/96, one
#                             per 512-wide m-chunk)
#   out[m,d] = colsum[m] * v[m,d]   (GpSimd tensor_scalar)
#
# Splitting exp halves the scalar-engine time vs the all-ACT version
# (~270us -> ~160us/core): ACT ~19.6us/head, DVE ~20.6us/head, PE ~20.5us/
# head all run nearly balanced.
#
# Sharding: 64 (b,h) pairs split across 8 cores, 8 pairs each (SPMD, no
# cross-core communication).  Q/K are pre-transposed on the host so the
# contraction dim (Dh=64) lands on SBUF partitions for the PE.

import os

import numpy as np

import concourse.mybir as mybir
import concourse.tile as tile
from concourse import bacc
from concourse.bass_utils import run_bass_kernel_spmd

B, H, N, D = 4, 16, 2048, 64
N_CORES = 8
H_LOC = (B * H) // N_CORES  # 8 (b,h) pairs per core
P = 128                     # partition tile along n
NT = N // P                 # 16 n-tiles
SCALE = 0.125               # (DIM // N_HEADS) ** -0.5
MH = 2                      # m processed in halves of 1024 (PSUM bank budget)
MW = N // MH                # 1024

# Degree-2 polynomial p with p(u)^8 ~ e^(8u) on |u| <= ~0.8, u = s/64
# (minimax fit of the composed approximation's relative error; |s| <= ~50
# in-distribution, tails to ~68 degrade gracefully).
_C = (1.00847688, 1.06738768, 0.48165367)
CF0 = float(_C[0])
CF1 = float(_C[1] / 64.0)
CF2 = float(_C[2] / (64.0 * 64.0))

# per-head tile types: A = ACT exp (accumulator rowsum), D = custom DVE exp
# (in-op accum rowsum).  Alternating keeps both engines fed within each
# 4-tile g-batch.
TILE_TYPES = "ADAD" "ADAD" "ADAD" "ADAD"

f32 = mybir.dt.float32
f32r = mybir.dt.float32r
bf16 = mybir.dt.bfloat16
Exp = mybir.ActivationFunctionType.Exp

_EXP_OP = None


def _get_exp_op():
    """Register the custom DVE op: out = (C0 + x(C1 + x C2))^8, accum=sum."""
    global _EXP_OP
    if _EXP_OP is None:
        from concourse.dve_spec import Spec, Src0, C0, C1, C2, sq, AluOp
        from concourse.dve_spec import lower as dve_lower
        from concourse.dve_spec import _has_src1
        from concourse.dve_ops import DveOp, OPS, get_dve_sub_opcode
        import concourse.dve_ops as dve_ops_mod
        from concourse.dve_uop import DveOpSpec

        poly = C0 + Src0 * (C1 + Src0 * C2)
        spec = Spec(body=sq(sq(sq(poly))), accum=AluOp.ADD)
        op = DveOp("EXP_POLY8_ANT", spec, subdim=False, uops_sha={})
        OPS.append(op)
        dve_ops_mod.CUSTOM_DVE_SPECS[op.name] = spec
        dve_ops_mod._SUB_OPCODE_FOR_NAME[op.name] = (
            dve_ops_mod._CUSTOM_DVE_ROW_BASE + len(OPS) - 1
        )
        for ver in ("v3", "v4"):
            op.uops_sha[ver] = DveOpSpec(
                name=op.name, opcode=get_dve_sub_opcode(op.name),
                uops=dve_lower(spec, ver=ver), rd1_en=_has_src1(spec),
            ).sha(ver)
        _EXP_OP = op
    return _EXP_OP


def _attention_kernel(tc, out, qT, kT, vin):
    nc = tc.nc
    exp_op = _get_exp_op()

    with (
        tc.tile_pool(name="qk", bufs=3) as qk_pool,
        tc.tile_pool(name="ev", bufs=14) as e_pool,
        tc.tile_pool(name="vo", bufs=4) as vo_pool,
        tc.tile_pool(name="st", bufs=2) as st_pool,
        tc.tile_pool(name="s_ps", bufs=3, space="PSUM") as s_pool,
        tc.tile_pool(name="c_ps", bufs=2, space="PSUM") as c_pool,
    ):
        # Preload the exp table set and start the PE p-state ramp while the
        # first DMAs are in flight.
        warm = st_pool.tile([P, 1], f32, tag="warm")
        nc.gpsimd.memset(warm[:, :], 0.0)
        nc.scalar.activation(warm[:, :], warm[:, :], func=Exp)
        warm_ps = c_pool.tile([1, 1], f32, tag="csum")
        nc.tensor.matmul(
            warm_ps[0:1, 0:1], lhsT=warm[0:1, 0:1], rhs=warm[0:1, 0:1],
            start=True, stop=True, skip_group_check=True,
        )

        # Q/K/V loads for head h, emitted one head ahead so the SP sequencer
        # issues them before it blocks on the previous head's tail DMAs.
        loaded = {}

        def emit_loads(h, first=False):
            q_s = qk_pool.tile([D, N], f32r, tag="q")
            k_s = qk_pool.tile([D, N], f32r, tag="k")
            if first:
                # order so the very first S fill's operands land earliest
                parts = [(k_s, kT, 0, 512), (q_s, qT, 0, 512),
                         (k_s, kT, 512, MW), (k_s, kT, MW, N),
                         (q_s, qT, 512, MW), (q_s, qT, MW, N)]
                for t_s, src, lo, hi in parts:
                    nc.sync.dma_start(out=t_s[:, lo:hi], in_=src[h, :, lo:hi])
            else:
                for half in range(2):
                    sl = slice(half * MW, (half + 1) * MW)
                    nc.sync.dma_start(out=k_s[:, sl], in_=kT[h, :, sl])
                    nc.sync.dma_start(out=q_s[:, sl], in_=qT[h, :, sl])
            v_s = vo_pool.tile([P, NT, D], f32, tag="v")
            nc.sync.dma_start(
                out=v_s[:, :, :], in_=vin[h].rearrange("(p t) d -> p t d", p=P)
            )
            # prepare the colsum accumulator a head ahead too, so its
            # memset (DVE) and PSUM WAR never stall the head boundary
            c_ps = c_pool.tile([P, 512], f32, tag="csum")
            nc.vector.memset(c_ps[:, :], 0.0)
            loaded[h] = (q_s, k_s, v_s, c_ps)

        emit_loads(0, first=True)

        # colsum matmuls pending emission: (min_slot, j, c_ps, g_bf, e_j, tail)
        pending = []
        slot = 0

        def emit_colsum(entry, c_lo, c_hi):
            _, j, c_ps, g_bf, e_j, tail_fn = entry
            for c in range(c_lo, c_hi):
                nc.tensor.matmul(
                    c_ps[32 * c : 32 * c + 1, :],
                    lhsT=g_bf[:, j : j + 1],
                    rhs=e_j[:, c * 512 : (c + 1) * 512],
                    start=(j == 0),
                    stop=(j == NT - 1),
                    skip_group_check=True,
                    tile_position=(0, 32 * c),
                )
            if c_hi == N // 512 and tail_fn is not None:
                tail_fn()

        for h in range(H_LOC):
            last_head = h == H_LOC - 1
            q_s, k_s, v_s, c_ps = loaded.pop(h)
            if not last_head:
                emit_loads(h + 1)

            # per-tile rowsum parts: [:, i, 0] and [:, i, 1] summed later
            rs_parts = st_pool.tile([P, NT, 2], f32, tag="rsp")
            rowsum = st_pool.tile([P, NT], f32, tag="rowsum")
            g = st_pool.tile([P, NT], f32, tag="g")
            g_bf = st_pool.tile([P, NT], bf16, tag="gbf")
            e_tiles = []

            def make_tail(h=h, c_ps=c_ps, v_s=v_s, last_head=last_head):
                def tail():
                    # colsum [4 x 512 at partitions 0/32/64/96] -> csT [P, NT].
                    # m = 16p + t, so chunk-major cs4 [4x512] and csT [128,16]
                    # walk m in the same order: one direct SBUF->SBUF DMA.
                    cs4 = st_pool.tile([P, 512], f32, tag="cs4")
                    nc.vector.tensor_copy(cs4[:, :], c_ps[:, :])
                    csT = st_pool.tile([P, NT], f32, tag="csT")
                    nc.sync.dma_start(out=csT[:, :], in_=cs4[0:P:32, :])
                    o_s = vo_pool.tile([P, NT, D], f32, tag="o")
                    eng = nc.vector if last_head else nc.gpsimd
                    out_r = out[h].rearrange("(p t) d -> p t d", p=P)
                    halves = ((0, NT // 2), (NT // 2, NT)) if last_head else ((0, NT),)
                    for t0, t1 in halves:
                        eng.tensor_tensor(
                            o_s[:, t0:t1, :],
                            v_s[:, t0:t1, :],
                            csT[:, t0:t1].unsqueeze(-1).broadcast_to((P, t1 - t0, D)),
                            op=mybir.AluOpType.mult,
                        )
                        nc.sync.dma_start(
                            out=out_r[:, t0:t1, :], in_=o_s[:, t0:t1, :]
                        )

                return tail

            tail_fn = make_tail()

            # g-batches: (first_tile, last_tile inclusive); the last head
            # finishes with micro-batches so colsum work drains in-loop
            if last_head:
                batches = [(0, 3), (4, 7), (8, 11), (12, 13), (14, 14), (15, 15)]
            else:
                batches = [(b0, b0 + 3) for b0 in range(0, NT, 4)]
            batch_of = {}
            for b0, b1 in batches:
                for j in range(b0, b1 + 1):
                    batch_of[j] = (b0, b1)

            for i in range(NT):
                slot += 1
                ttype = TILE_TYPES[i]
                e_i = e_pool.tile([P, N], bf16, tag="e")
                e_tiles.append(e_i)
                # drip-feed pending colsum matmuls: half a tile's worth
                # after each m-half, so PE bursts never delay the S fills
                entry = None
                for mh in range(MH):
                    s_ps = s_pool.tile([P, MW], f32, tag="s")
                    for c in range(MW // 512):
                        m0 = mh * MW + c * 512
                        nc.tensor.matmul(
                            s_ps[:, c * 512 : (c + 1) * 512],
                            lhsT=q_s[:, i * P : (i + 1) * P],
                            rhs=k_s[:, m0 : m0 + 512],
                            start=True,
                            stop=True,
                        )
                    if ttype == "A":
                        nc.scalar.activation(
                            e_i[:, mh * MW : (mh + 1) * MW],
                            s_ps[:, :],
                            func=Exp,
                            scale=SCALE,
                            accum_out=rs_parts[:, i, mh : mh + 1],
                        )
                    else:
                        bi = nc.vector._custom_dve(
                            exp_op,
                            out=e_i[:, mh * MW : (mh + 1) * MW],
                            in0=s_ps[:, :],
                            s0=CF0, s1=CF1, imm2=CF2,
                            accum_out=rs_parts[:, i, mh : mh + 1],
                        )
                        bi.perf_max = 2
                    if mh == 0:
                        if pending and pending[0][0] <= slot:
                            entry = pending.pop(0)
                        if entry is not None:
                            emit_colsum(entry, 0, 2)
                    elif entry is not None:
                        emit_colsum(entry, 2, 4)

                if i == batch_of[i][1]:  # batch boundary: g for the batch
                    b0, b1 = batch_of[i]
                    sl = slice(b0, b1 + 1)
                    nc.vector.tensor_tensor(
                        rowsum[:, sl],
                        rs_parts[:, sl, 0],
                        rs_parts[:, sl, 1],
                        op=mybir.AluOpType.add,
                    )
                    nc.vector.reciprocal(g[:, sl], rowsum[:, sl])
                    nc.vector.tensor_copy(g_bf[:, sl], g[:, sl])
                    lag = 1
                    for idx, j in enumerate(range(b0, b1 + 1)):
                        pending.append(
                            (
                                slot + lag + idx,
                                j,
                                c_ps,
                                g_bf,
                                e_tiles[j],
                                tail_fn if j == NT - 1 else None,
                            )
                        )

            if last_head:
                while pending:
                    emit_colsum(pending.pop(0), 0, N // 512)


_NC_CACHE = None


def _get_nc():
    global _NC_CACHE
    if _NC_CACHE is None:
        nc = bacc.Bacc("TRN2", target_bir_lowering=False, debug=False)
        qT = nc.dram_tensor("qT", [H_LOC, D, N], f32r, kind="ExternalInput").ap()
        kT = nc.dram_tensor("kT", [H_LOC, D, N], f32r, kind="ExternalInput").ap()
        vin = nc.dram_tensor("v", [H_LOC, N, D], f32, kind="ExternalInput").ap()
        out = nc.dram_tensor("out", [H_LOC, N, D], f32, kind="ExternalOutput").ap()
        with tile.TileContext(nc) as tc:
            _attention_kernel(tc, out, qT, kT, vin)
        nc.compile()
        # perf_max set pre-compile on the BassInstruction wrappers does not
        # survive scheduling; re-apply on the compiled stream.
        fn = nc.m.functions[0]
        for inst in [i for b in fn.blocks for i in b.instructions]:
            if getattr(inst, "op_name", None) == "EXP_POLY8_ANT":
                inst.perf_max = 2
        _NC_CACHE = nc
    return _NC_CACHE


def kernel(q, k, v):
    q = np.asarray(q, dtype=np.float32).reshape(B * H, N, D)
    k = np.asarray(k, dtype=np.float32).reshape(B * H, N, D)
    v = np.asarray(v, dtype=np.float32).reshape(B * H, N, D)

    in_maps = []
    for c in range(N_CORES):
        sl = slice(H_LOC * c, H_LOC * (c + 1))
        in_maps.append(
            {
                "qT": np.ascontiguousarray(q[sl].transpose(0, 2, 1)),
                "kT": np.ascontiguousarray(k[sl].transpose(0, 2, 1)),
                "v": np.ascontiguousarray(v[sl]),
            }
        )

    trace = bool(os.environ.get("KERNEL_TRACE"))
    res = run_bass_kernel_spmd(
        _get_nc(), in_maps, core_ids=list(range(N_CORES)), trace=trace
    )
    if trace:
        print(f"HW exec time: {res.exec_time_ns} ns")
        if res.instructions_and_trace is not None:
            print(f"trace: {res.instructions_and_trace[1]}")

    outs = [r["out"] for r in res.results]
    return np.concatenate(outs, axis=0).reshape(B, H, N, D)


# revision 7
# speedup vs baseline: 1.4647x; 1.2915x over previous
# Trainium2 Bass kernel for nn_MultiHeadAttention_48533130445634 — v3.
#
# Math (faithful to the reference, including its unusual second einsum):
#   scores[b,h,n,m] = softmax_m( (q[b,h,n,:] . k[b,h,m,:]) * 0.125 )
#   out[b,h,m,d]    = (sum_n scores[b,h,n,m]) * v[b,h,m,d]
#
# out = V * colsum(softmax).  Per (b,h), tiled over n (128 rows):
#   S_i = Q_i K^T                 (PE, f32r, PSUM, 1024-wide halves)
#   E_i = exp(S_i*0.125)*2^-5     (fp8e4m3 out; the roofline, split:
#                                  even tiles on ACT (native Exp + rowsum
#                                  via the ACT accumulator), odd tiles on
#                                  the DVE via a custom op p(s)^8 with a
#                                  fitted degree-2 p + in-op accum rowsum)
#   g~_j = GS / rowsum_j          (DVE recip; stored fp8, stride-16 pairs)
#   colsumT[:, t] += E_pair[:,:,128t:..].T @ g~_pair
#                                 (PE fp8 DoubleRow: the whole head's
#                                  colsum accumulates into ONE [128,16]
#                                  PSUM tile at dst partition 0; m-index
#                                  lands on partitions as m = 128 t + p)
#   out[m,d] = colsumT[m] * (v[m,d]/GS)   (GpSimd; V pre-scaled on host)
#
# The exp split halves the scalar-engine wall (~272 -> ~160 us/core); the
# transposed fp8-DoubleRow colsum removes the old 109 us/core colsum matmul
# stream (output free-size 1 per accumulation step) and the cs4 gather.
# The poly approximation's common-mode error cancels in softmax's ratio and
# the colsum averages the rest (end-to-end rel err ~3e-3 incl fp8 E/g).
#
# Sharding: 64 (b,h) pairs across 8 cores, 8 each (SPMD, no cross-core
# comm).  Q/K host-transposed so Dh lands on partitions; V/out use the
# m = 128 t + p layout matching colsumT.

import math
import os

import numpy as np

import concourse.mybir as mybir
import concourse.tile as tile
from concourse import bacc
from concourse.bass_utils import run_bass_kernel_spmd

B, H, N, D = 4, 16, 2048, 64
N_CORES = 8
H_LOC = (B * H) // N_CORES
P = 128
NT = N // P                 # 16 n-tiles; also 16 m-chunks of 128
NP = NT // 2                # 8 tile pairs
SCALE = 0.125
MH = 2
MW = N // MH                # 1024

ESCALE = 2.0 ** -5          # E stored as E*2^-5: fp8e4m3 (max 240) safe
GS = 4096.0                 # g~ = GS/rowsum' in fp8; undone via V/GS on host

# p(u)^8 ~ e^(8u) on |u| <= ~0.8 (u = s/64); coefficients additionally
# fold ESCALE^(1/8) so the op emits e^(s/8)*ESCALE directly.
_C = (1.00847688, 1.06738768, 0.48165367)
_ES = ESCALE ** (1.0 / 8.0)
CF0 = float(_C[0] * _ES)
CF1 = float(_C[1] * _ES / 64.0)
CF2 = float(_C[2] * _ES / (64.0 * 64.0))

TILE_TYPES = "ADAD" "ADAD" "ADAD" "ADAD"  # A = ACT exp, D = DVE custom exp

f32 = mybir.dt.float32
f32r = mybir.dt.float32r
f8 = mybir.dt.float8e4
Exp = mybir.ActivationFunctionType.Exp

_EXP_OP = None


def _get_exp_op():
    """Custom DVE op: out = (C0 + x(C1 + x C2))^8, accum_out = row sum."""
    global _EXP_OP
    if _EXP_OP is None:
        from concourse.dve_spec import Spec, Src0, C0, C1, C2, sq, AluOp
        from concourse.dve_spec import lower as dve_lower
        from concourse.dve_spec import _has_src1
        from concourse.dve_ops import DveOp, OPS, get_dve_sub_opcode
        import concourse.dve_ops as dve_ops_mod
        from concourse.dve_uop import DveOpSpec

        poly = C0 + Src0 * (C1 + Src0 * C2)
        spec = Spec(body=sq(sq(sq(poly))), accum=AluOp.ADD)
        op = DveOp("EXP_POLY8_ANT", spec, subdim=False, uops_sha={})
        OPS.append(op)
        dve_ops_mod.CUSTOM_DVE_SPECS[op.name] = spec
        dve_ops_mod._SUB_OPCODE_FOR_NAME[op.name] = (
            dve_ops_mod._CUSTOM_DVE_ROW_BASE + len(OPS) - 1
        )
        for ver in ("v3", "v4"):
            op.uops_sha[ver] = DveOpSpec(
                name=op.name, opcode=get_dve_sub_opcode(op.name),
                uops=dve_lower(spec, ver=ver), rd1_en=_has_src1(spec),
            ).sha(ver)
        _EXP_OP = op
    return _EXP_OP


def _attention_kernel(tc, out, qT, kT, vin):
    nc = tc.nc
    exp_op = _get_exp_op()

    with (
        tc.tile_pool(name="qk", bufs=3) as qk_pool,
        tc.tile_pool(name="ev", bufs=2) as e_pool,
        tc.tile_pool(name="vo", bufs=4) as vo_pool,
        tc.tile_pool(name="st", bufs=2) as st_pool,
        tc.tile_pool(name="s_ps", bufs=3, space="PSUM") as s_pool,
        tc.tile_pool(name="c_ps", bufs=2, space="PSUM") as c_pool,
    ):
        # Exp table preload + PE p-state ramp while the first DMAs land.
        warm = st_pool.tile([P, 1], f32, tag="warm")
        nc.gpsimd.memset(warm[:, :], 0.0)
        nc.scalar.activation(warm[:, :], warm[:, :], func=Exp)
        warm_ps = c_pool.tile([P, NT], f32, tag="csum")
        nc.tensor.matmul(
            warm_ps[0:1, 0:1], lhsT=warm[0:1, 0:1], rhs=warm[0:1, 0:1],
            start=True, stop=True, skip_group_check=True,
        )
        bias_t = st_pool.tile([P, 1], f32, tag="bias")
        nc.gpsimd.memset(bias_t[:, :], float(math.log(ESCALE)))
        # single-row zeros: lhsT/rhs of the c_psT-clearing matmul (start=True
        # zero-marking is bank-row-wide, so clear the whole [P, NT] region
        # with one matmul instead of per-column starts)
        zrow = st_pool.tile([1, P + NT], mybir.dt.bfloat16, tag="zrow")
        nc.vector.memset(zrow[:, :], 0.0)

        loaded = {}

        def emit_loads(h, first=False):
            q_s = qk_pool.tile([D, N], f32r, tag="q")
            k_s = qk_pool.tile([D, N], f32r, tag="k")
            if first:
                parts = [(k_s, kT, 0, 512), (q_s, qT, 0, 512),
                         (k_s, kT, 512, MW), (k_s, kT, MW, N),
                         (q_s, qT, 512, MW), (q_s, qT, MW, N)]
                for t_s, src, lo, hi in parts:
                    nc.sync.dma_start(out=t_s[:, lo:hi], in_=src[h, :, lo:hi])
            else:
                for half in range(2):
                    sl = slice(half * MW, (half + 1) * MW)
                    nc.sync.dma_start(out=k_s[:, sl], in_=kT[h, :, sl])
                    nc.sync.dma_start(out=q_s[:, sl], in_=qT[h, :, sl])
            # V in the m = 128 t + p layout: v_s[p, t, d] = v[128 t + p, d]
            v_s = vo_pool.tile([P, NT, D], f32, tag="v")
            nc.sync.dma_start(
                out=v_s[:, :, :], in_=vin[h].rearrange("(t p) d -> p t d", p=P)
            )
            e_pairs = [
                e_pool.tile([P, 2, N], f8, tag=f"ep{jp}", name=f"ep{jp}_{h}")
                for jp in range(NP)
            ]
            c_psT = c_pool.tile([P, NT], f32, tag="csum")
            nc.tensor.matmul(
                c_psT[:, :], lhsT=zrow[:, 0:P], rhs=zrow[:, P : P + NT],
                start=True, stop=True, skip_group_check=True,
            )
            loaded[h] = (q_s, k_s, v_s, e_pairs, c_psT)

        emit_loads(0, first=True)

        # pending colsum pair contributions: (min_slot, jp, c_psT, g8, e_pair,
        # tail)
        pending = []
        slot = 0

        def emit_colsum(entry, t_lo, t_hi):
            _, jp, c_psT, g8, e_pair, tail_fn = entry
            for t in range(t_lo, t_hi):
                nc.tensor.matmul(
                    c_psT[:, t : t + 1],
                    lhsT=e_pair[:, :, 128 * t : 128 * (t + 1)],
                    rhs=g8[:, :, jp : jp + 1],
                    start=False,
                    stop=(jp == NP - 1),
                    skip_group_check=True,
                    perf_mode=mybir.MatmulPerfMode.DoubleRow,
                )
            if t_hi == NT and tail_fn is not None:
                tail_fn()

        for h in range(H_LOC):
            last_head = h == H_LOC - 1
            q_s, k_s, v_s, e_pairs, c_psT = loaded.pop(h)
            if not last_head:
                emit_loads(h + 1)

            # rowsum parts in pair layout: [:, i%2, i//2, mh]
            rs_t = st_pool.tile([P, 2, NP, 2], f32, tag="rsp")
            rowsum_t = st_pool.tile([P, 2, NP], f32, tag="rowsum")
            g32_t = st_pool.tile([P, 2, NP], f32, tag="g32")
            g8_t = st_pool.tile([P, 2, NP], f8, tag="g8")

            def make_tail(h=h, c_psT=c_psT, v_s=v_s, last_head=last_head):
                def tail():
                    cs_sb = st_pool.tile([P, NT], f32, tag="cs")
                    nc.vector.tensor_copy(cs_sb[:, :], c_psT[:, :])
                    o_s = vo_pool.tile([P, NT, D], f32, tag="o")
                    eng = nc.vector if last_head else nc.gpsimd
                    out_r = out[h].rearrange("(t p) d -> p t d", p=P)
                    halves = ((0, NT // 2), (NT // 2, NT)) if last_head else ((0, NT),)
                    for t0, t1 in halves:
                        eng.tensor_tensor(
                            o_s[:, t0:t1, :],
                            v_s[:, t0:t1, :],
                            cs_sb[:, t0:t1].unsqueeze(-1).broadcast_to((P, t1 - t0, D)),
                            op=mybir.AluOpType.mult,
                        )
                        nc.sync.dma_start(
                            out=out_r[:, t0:t1, :], in_=o_s[:, t0:t1, :]
                        )

                return tail

            tail_fn = make_tail()

            if last_head:
                batches = [(0, 3), (4, 7), (8, 11), (12, 13), (14, 15)]
            else:
                batches = [(b0, b0 + 3) for b0 in range(0, NT, 4)]
            batch_of = {}
            for b0, b1 in batches:
                for j in range(b0, b1 + 1):
                    batch_of[j] = (b0, b1)

            for i in range(NT):
                slot += 1
                ttype = TILE_TYPES[i]
                e_slot_pair = e_pairs[i // 2]
                entry = None
                for mh in range(MH):
                    s_ps = s_pool.tile([P, MW], f32, tag="s")
                    for c in range(MW // 512):
                        m0 = mh * MW + c * 512
                        nc.tensor.matmul(
                            s_ps[:, c * 512 : (c + 1) * 512],
                            lhsT=q_s[:, i * P : (i + 1) * P],
                            rhs=k_s[:, m0 : m0 + 512],
                            start=True,
                            stop=True,
                        )
                    if ttype == "A":
                        nc.scalar.activation(
                            e_slot_pair[:, i % 2, mh * MW : (mh + 1) * MW],
                            s_ps[:, :],
                            func=Exp,
                            scale=SCALE,
                            bias=bias_t[:, :],
                            accum_out=rs_t[:, i % 2, i // 2, mh : mh + 1],
                        )
                    else:
                        nc.vector._custom_dve(
                            exp_op,
                            out=e_slot_pair[:, i % 2, mh * MW : (mh + 1) * MW],
                            in0=s_ps[:, :],
                            s0=CF0, s1=CF1, imm2=CF2,
                            accum_out=rs_t[:, i % 2, i // 2, mh : mh + 1],
                        )
                    # drip-feed pending colsum pairs, half a pair per m-half
                    if mh == 0:
                        if pending and pending[0][0] <= slot:
                            entry = pending.pop(0)
                        if entry is not None:
                            emit_colsum(entry, 0, NT // 2)
                    elif entry is not None:
                        emit_colsum(entry, NT // 2, NT)

                if i == batch_of[i][1]:  # batch boundary: g for the batch
                    b0, b1 = batch_of[i]
                    jp0, jp1 = b0 // 2, b1 // 2 + 1  # pair range
                    sl = slice(jp0, jp1)
                    nc.vector.tensor_tensor(
                        rowsum_t[:, :, sl],
                        rs_t[:, :, sl, 0],
                        rs_t[:, :, sl, 1],
                        op=mybir.AluOpType.add,
                    )
                    nc.vector.reciprocal(g32_t[:, :, sl], rowsum_t[:, :, sl])
                    nc.vector.tensor_scalar(
                        out=g8_t[:, :, sl], in0=g32_t[:, :, sl],
                        scalar1=GS, scalar2=None,
                        op0=mybir.AluOpType.mult,
                    )
                    lag = 1
                    for idx, jp in enumerate(range(jp0, jp1)):
                        pending.append(
                            (
                                slot + lag + idx,
                                jp,
                                c_psT,
                                g8_t,
                                e_pairs[jp],
                                tail_fn if jp == NP - 1 else None,
                            )
                        )

            if last_head:
                while pending:
                    emit_colsum(pending.pop(0), 0, NT)


_NC_CACHE = None


def _get_nc():
    global _NC_CACHE
    if _NC_CACHE is None:
        nc = bacc.Bacc("TRN2", target_bir_lowering=False, debug=False)
        qT = nc.dram_tensor("qT", [H_LOC, D, N], f32r, kind="ExternalInput").ap()
        kT = nc.dram_tensor("kT", [H_LOC, D, N], f32r, kind="ExternalInput").ap()
        vin = nc.dram_tensor("v", [H_LOC, N, D], f32, kind="ExternalInput").ap()
        out = nc.dram_tensor("out", [H_LOC, N, D], f32, kind="ExternalOutput").ap()
        with tile.TileContext(nc) as tc:
            _attention_kernel(tc, out, qT, kT, vin)
        nc.compile()
        # custom-DVE fast-mode flag must be applied to the compiled stream
        fn = nc.m.functions[0]
        for inst in [i for b in fn.blocks for i in b.instructions]:
            if getattr(inst, "op_name", None) == "EXP_POLY8_ANT":
                inst.perf_max = 2
        _NC_CACHE = nc
    return _NC_CACHE


def kernel(q, k, v):
    q = np.asarray(q, dtype=np.float32).reshape(B * H, N, D)
    k = np.asarray(k, dtype=np.float32).reshape(B * H, N, D)
    v = np.asarray(v, dtype=np.float32).reshape(B * H, N, D)
    v_scaled = (v * (1.0 / GS)).astype(np.float32)

    in_maps = []
    for c in range(N_CORES):
        sl = slice(H_LOC * c, H_LOC * (c + 1))
        in_maps.append(
            {
                "qT": np.ascontiguousarray(q[sl].transpose(0, 2, 1)),
                "kT": np.ascontiguousarray(k[sl].transpose(0, 2, 1)),
                "v": np.ascontiguousarray(v_scaled[sl]),
            }
        )

    trace = bool(os.environ.get("KERNEL_TRACE"))
    res = run_bass_kernel_spmd(
        _get_nc(), in_maps, core_ids=list(range(N_CORES)), trace=trace
    )
    if trace:
        print(f"HW exec time: {res.exec_time_ns} ns")
        if res.instructions_and_trace is not None:
            print(f"trace: {res.instructions_and_trace[1]}")

    outs = [r["out"] for r in res.results]
    return np.concatenate(outs, axis=0).reshape(B, H, N, D)
